# revision 51
# baseline (speedup 1.0000x reference)
"""Trainium2 Bass kernel for nn_BitSpikeMambaModel (embed -> bitlinear x2 -> LN -> bitlinear head).

Self-contained: hardcodes shapes from the problem spec.

Sharding: data-parallel trunk over the 4096 tokens (512/core); the head
BitLinear is tensor-parallel over the vocab dim: head_w is zero-padded to
32768 rows and each core owns a 4096-row slice.  LN outputs are AllGathered
(fp16) so every core computes its vocab slice for all 4096 tokens.
Everything runs in single fp16 (no hi/lo); absmax-rel err ~5e-4 vs the 2e-2
gate.

Key perf choices:
  - all weight tensors are PRE-TILED on the host so each streamed chunk is a
    single contiguous DRAM region (16KB per-partition DMA packets, not 1KB)
  - BitNet ternary quantization on device (DVE): q = 1{w>.5s} - 1{w<-.5s}
    == clip(round_half_even(w/s),-1,1).  Scales reduced from fp16 copies
    (flat contiguous chunks); head scale AllReduced across cores.  The
    quantize compare runs on fp32 weights for exactness.
  - head: 2 token-halves; per half the 4 gathered 512-token blocks are SBUF
    resident; each 256-col weight chunk feeds dt-outer interleaved PSUM
    chains (4 chains in flight) to keep the PE dense (HAM warm)
  - LayerNorm stats via ones-matmul on the tensor engine, Newton rsqrt
Output per core: [32, 2, 128, 2048] fp16 blocks; host reassembles to
[2, S, V] fp32 (drops the 768 vocab pad rows).
"""

import numpy as np

import concourse.bass as bass
import concourse.bacc as bacc
import concourse.mybir as mybir
import concourse.tile as tile
from concourse.bass_utils import run_bass_kernel_spmd

F32 = mybir.dt.float32
F16 = mybir.dt.float16
BF16 = mybir.dt.bfloat16
I16 = mybir.dt.int16
AF = mybir.ActivationFunctionType
OP = mybir.AluOpType
AX = mybir.AxisListType

VOCAB = 32000
DIM = 2048
BATCH = 2
SEQ = 2048
NCORES = 8
EPS = 1e-5
VPAD = 32768  # vocab padded to a multiple of 8*128


class Cfg:
    def __init__(self, ncores=NCORES, adt="f16", halves=2, identity_ln=False):
        self.ncores = ncores
        self.identity_ln = identity_ln  # gamma==1, beta==0 (host-verified)
        self.V, self.D = VOCAB, DIM
        self.T = (BATCH * SEQ) // ncores      # own tokens per core (512)
        self.VS = VPAD // ncores              # head slice rows per core (4096)
        self.DT = DIM // 128                  # d-tiles (16)
        self.NO_TR = DIM // 128               # trunk output tiles (16)
        self.NO_HD = self.VS // 128           # head output tiles per core (32)
        self.adt = {"f16": F16, "bf16": BF16}[adt]
        self.halves = halves                  # head token-halves


def build(cfg: Cfg):
    D, T, DT = cfg.D, cfg.T, cfg.DT
    NBLK = cfg.ncores                 # gathered 512-token blocks
    BPH = NBLK // cfg.halves          # blocks per half (4)
    HT = BPH * T                      # tokens per half (2048)
    ADT = cfg.adt
    NCH_TR = cfg.NO_TR // 2           # 256-col chunks per trunk layer (8)
    NCH_HD = cfg.NO_HD // 2           # 256-col chunks for the head (16)
    SC = 4096                         # f16 scale-pass flat chunk columns
    nc = bacc.Bacc("TRN2", target_bir_lowering=False, debug=False,
                   num_devices=cfg.ncores)

    # ---- DRAM I/O (weights pre-tiled: [chunk, 128, DT, 256]) ----
    idx_d = nc.dram_tensor("idx", [128, T // 16], I16, kind="ExternalInput")
    embh_d = nc.dram_tensor("embh", [VOCAB, D], ADT, kind="ExternalInput")
    w0t_d = nc.dram_tensor("w0t", [NCH_TR, 128, DT, 256], F32, kind="ExternalInput")
    w1t_d = nc.dram_tensor("w1t", [NCH_TR, 128, DT, 256], F32, kind="ExternalInput")
    hwt_d = nc.dram_tensor("hwt", [NCH_HD, 128, DT, 256], F32, kind="ExternalInput")
    w0h_d = nc.dram_tensor("w0h", [D * D // (128 * SC), 128, SC], F16,
                           kind="ExternalInput")
    w1h_d = nc.dram_tensor("w1h", [D * D // (128 * SC), 128, SC], F16,
                           kind="ExternalInput")
    hwh_d = nc.dram_tensor("hwh", [D * cfg.VS // (128 * SC), 128, SC], F16,
                           kind="ExternalInput")
    b0_d = nc.dram_tensor("b0r", [128, DT], F32, kind="ExternalInput")
    b1_d = nc.dram_tensor("b1r", [128, DT], F32, kind="ExternalInput")
    gam_d = nc.dram_tensor("gamr", [128, DT], F32, kind="ExternalInput")
    bet_d = nc.dram_tensor("betr", [128, DT], F32, kind="ExternalInput")
    hb_d = nc.dram_tensor("hbr", [128, cfg.NO_HD], F32, kind="ExternalInput")
    # [ot, token-half, p, block*256+t]; block column j = global (pid+j)%8
    out_d = nc.dram_tensor("out", [cfg.NO_HD, 2, 128, NBLK * (T // 2)], F16,
                           kind="ExternalOutput")

    with tile.TileContext(nc) as tc:
        import contextlib
        with contextlib.ExitStack() as ctx:
            cst = ctx.enter_context(tc.tile_pool(name="cst", bufs=1))
            act = ctx.enter_context(tc.tile_pool(name="act", bufs=2))
            blk = ctx.enter_context(tc.tile_pool(name="blk", bufs=2))
            sstr = ctx.enter_context(tc.tile_pool(name="sstr", bufs=2))
            wstr = ctx.enter_context(tc.tile_pool(name="wstr", bufs=2))
            qbuf = ctx.enter_context(tc.tile_pool(name="qbuf", bufs=2))
            mbuf = ctx.enter_context(tc.tile_pool(name="mbuf", bufs=1))
            evt = ctx.enter_context(tc.tile_pool(name="evt", bufs=2))
            osb = ctx.enter_context(tc.tile_pool(name="osb", bufs=2))
            sml = ctx.enter_context(tc.tile_pool(name="sml", bufs=1))
            scl = ctx.enter_context(tc.tile_pool(name="scl", bufs=1))
            ps_mm = ctx.enter_context(tc.tile_pool(name="ps_mm", bufs=5, space="PSUM"))
            ps_st = ctx.enter_context(tc.tile_pool(name="ps_st", bufs=1, space="PSUM"))
            drp = ctx.enter_context(tc.tile_pool(name="drp", bufs=1, space="DRAM"))

            # ---- constants ----
            ones_col = cst.tile([128, 1], ADT)
            nc.any.memset(ones_col[:], 1.0)
            ones_row = cst.tile([1, 128], F32)
            nc.any.memset(ones_row[:], 1.0)
            idx_sb = cst.tile([128, T // 16], I16)
            nc.sync.dma_start(idx_sb[:], idx_d.ap())
            b0s = cst.tile([128, DT], F32)
            nc.sync.dma_start(b0s[:], b0_d.ap())
            b1s = cst.tile([128, DT], F32)
            nc.sync.dma_start(b1s[:], b1_d.ap())
            gams = cst.tile([128, DT], F32)
            nc.sync.dma_start(gams[:], gam_d.ap())
            bets = cst.tile([128, DT], F32)
            nc.sync.dma_start(bets[:], bet_d.ap())
            hbs = cst.tile([128, cfg.NO_HD], F32)
            nc.sync.dma_start(hbs[:], hb_d.ap())

            # ---- abs-sum of a flat-tiled fp16 [nch, 128, SC] tensor.
            # DMAs ride the scalar HWDGE ring (so weight chunks on the sync
            # ring are never queued behind them); per-chunk reduces are
            # emitted as "side jobs" interleaved into the trunk loops so the
            # in-order DVE is not blocked up front.
            def abs_sum_jobs(ten_d, nch, tagsuf):
                c1 = SC // (16 * 128)
                part = scl.tile([128, c1 * nch], F32, tag=f"part{tagsuf}",
                                name=f"part{tagsuf}")
                tiles = {}

                def dma(ch):
                    wt = sstr.tile([128, SC], F16, tag="sstr", name=f"sw{tagsuf}")
                    nc.scalar.dma_start(wt[:], ten_d.ap()[ch])
                    tiles[ch] = wt

                def red(ch):
                    # stage 1: 16-deep f16 partials at 2x DVE rate (chains are
                    # short so f16 accumulation error is negligible)
                    st1 = sml.tile([128, SC // 16], F16, tag="st1", name="st1")
                    with nc.allow_low_precision(reason="16-deep |w| partials"):
                        nc.vector.tensor_reduce(
                            st1[:],
                            tiles.pop(ch)[:].rearrange("p (c k) -> p c k", k=16),
                            axis=AX.X, op=OP.add, apply_absolute_value=True)
                    nc.vector.tensor_reduce(
                        part[:, ch * c1:(ch + 1) * c1],
                        st1[:].rearrange("p (c1 k) -> p c1 k", k=128),
                        axis=AX.X, op=OP.add)

                def job(ch):
                    if ch + 1 < nch:
                        dma(ch + 1)
                    red(ch)

                dma(0)
                return part, [lambda ch=ch: job(ch) for ch in range(nch)]

            def abs_total(part, tagsuf):
                p3 = sml.tile([128, 1], F32, tag=f"p3{tagsuf}", name=f"p3{tagsuf}")
                nc.vector.tensor_reduce(p3[:], part[:], axis=AX.X, op=OP.add)
                tps = ps_st.tile([1, 1], F32, tag="pa")
                ocol32 = sml.tile([128, 1], F32, tag="ones32", name="ocol32")
                nc.any.memset(ocol32[:], 1.0)
                nc.tensor.matmul(tps[:], ocol32[:], p3[:], start=True, stop=True)
                tot = sml.tile([1, 1], F32, tag=f"tot{tagsuf}", name=f"tot{tagsuf}")
                nc.scalar.activation(tot[:], tps[:], AF.Copy)
                return tot

            # scalar [1,1] -> replicated [128,1] * mul; s=max(s,EPS), h=.5s, nh=-.5s
            def finalize_scale(tot, mul, tagsuf):
                rps = ps_st.tile([128, 1], F32, tag="pa")
                nc.tensor.matmul(rps[:], ones_row[:], tot[:], start=True, stop=True)
                s = scl.tile([128, 1], F32, tag=f"s{tagsuf}")
                nc.scalar.activation(s[:], rps[:], AF.Copy, scale=mul)
                nc.vector.tensor_scalar(s[:], s[:], EPS, None, OP.max)
                h = scl.tile([128, 1], F32, tag=f"h{tagsuf}")
                nc.vector.tensor_scalar(h[:], s[:], 0.5, None, OP.mult)
                nh = scl.tile([128, 1], F32, tag=f"nh{tagsuf}")
                nc.vector.tensor_scalar(nh[:], h[:], -1.0, None, OP.mult)
                return s, h, nh

            # ---- embedding gather (2 queues) + all scale passes up front ----
            xt = act.tile([128, DT, T], ADT, tag="act")
            nc.gpsimd.dma_gather(out_ap=xt[:], in_ap=embh_d.ap(), idxs_ap=idx_sb[:],
                                 num_idxs=T, num_idxs_reg=T, elem_size=D,
                                 transpose=True)

            # w0 scale fully up front (s0 gates L0); w1/head scale reduces
            # are deferred into the L0/L1 loops as side jobs.
            part0, jobs0 = abs_sum_jobs(w0h_d, D * D // (128 * SC), "w0")
            for j in jobs0:
                j()
            tot0 = abs_total(part0, "w0")
            s0, h0, nh0 = finalize_scale(tot0, 1.0 / (D * D), "w0")
            part1, jobs1 = abs_sum_jobs(w1h_d, D * D // (128 * SC), "w1")
            parth, jobsh = abs_sum_jobs(hwh_d, D * cfg.VS // (128 * SC), "hd")

            # split a big DMA across both HWDGE rings (sync + scalar): one
            # ring delivers ~192GB/s; two in parallel halve chunk latency
            def dma2(dst, src, axis_half):
                h = axis_half
                nc.sync.dma_start(dst[:, :h], src[:, :h])
                nc.scalar.dma_start(dst[:, h:], src[:, h:])

            # ---- quantize one fp32 [128, DT, 256] chunk -> ternary ADT.
            # q = 1{w>h} - 1{w<-h} in two DVE ops:
            #   msk = (w >= -h) - 1            in {0,-1}
            #   q   = (w > h) + msk            in {1,0,-1}
            def quantize(wt, h_ap, nh_ap):
                msk = mbuf.tile([128, DT, 256], ADT, tag="msk")
                nc.vector.tensor_scalar(msk[:], wt, nh_ap[:], -1.0, OP.is_ge, OP.add)
                sgn = qbuf.tile([128, DT, 256], ADT, tag="sgn")
                nc.vector.scalar_tensor_tensor(sgn[:], wt, h_ap[:], msk[:],
                                               OP.is_gt, OP.add)
                return sgn

            # ---- trunk bitlinear: stream w, quantize, chains of DT matmuls ----
            def trunk_layer(wten, h_ap, nh_ap, rhs, consume, side_jobs=()):
                side = list(side_jobs)
                per = (len(side) + NCH_TR - 1) // NCH_TR if side else 0
                for g in range(NCH_TR):
                    wt = wstr.tile([128, DT, 256], F32, tag="wstr")
                    dma2(wt[:], wten.ap()[g], DT // 2)
                    q = quantize(wt[:], h_ap, nh_ap)
                    for _ in range(per):
                        if side:
                            side.pop(0)()
                    for j in range(2):
                        ot = g * 2 + j
                        pt = ps_mm.tile([128, T], F32, tag="ps_mm")
                        for dt in range(DT):
                            nc.tensor.matmul(pt[:], q[:, dt, j * 128:(j + 1) * 128],
                                             rhs[:, dt, :],
                                             start=(dt == 0), stop=(dt == DT - 1))
                        consume(ot, pt)

            # ---- layer 0 ----
            h1 = act.tile([128, DT, T], ADT, tag="act")

            def consume_l0(ot, pt):
                nc.scalar.activation(h1[:, ot, :], pt[:], AF.Identity,
                                     bias=b0s[:, ot:ot + 1], scale=s0[:])

            trunk_layer(w0t_d, h0, nh0, xt, consume_l0,
                        side_jobs=jobs1 + jobsh)
            tot1 = abs_total(part1, "w1")
            s1, h1s, nh1 = finalize_scale(tot1, 1.0 / (D * D), "w1")
            # head-scale AllReduce fires here so it completes during L1
            toth = abs_total(parth, "hd")
            bin_t = drp.tile([1, 1], F32, tag="cc_in")
            bout_t = drp.tile([1, 1], F32, tag="cc_out")
            nc.sync.dma_start(bin_t[:], toth[:])
            nc.gpsimd.collective_compute(
                "AllReduce", OP.add,
                replica_groups=[list(range(cfg.ncores))],
                ins=[bin_t[:].opt()], outs=[bout_t[:].opt()])
            toth_g = sml.tile([1, 1], F32, tag="tothg")
            nc.sync.dma_start(toth_g[:], bout_t[:])

            # ---- layer 1 (+ LN stats via ones-matmuls) ----
            y1 = act.tile([128, DT, T], ADT, tag="act")
            ps_s = ps_st.tile([1, T], F32, tag="ps_s")
            ps_q = ps_st.tile([1, T], F32, tag="ps_q")

            def consume_l1(ot, pt):
                nc.scalar.activation(y1[:, ot, :], pt[:], AF.Identity,
                                     bias=b1s[:, ot:ot + 1], scale=s1[:])
                sq = evt.tile([128, T], ADT, tag="evt16")
                nc.vector.tensor_tensor(sq[:], y1[:, ot, :], y1[:, ot, :], OP.mult)
                nc.tensor.matmul(ps_s[:], ones_col[:], y1[:, ot, :],
                                 start=(ot == 0), stop=(ot == DT - 1))
                nc.tensor.matmul(ps_q[:], ones_col[:], sq[:],
                                 start=(ot == 0), stop=(ot == DT - 1))

            trunk_layer(w1t_d, h1s, nh1, h1, consume_l1)

            # head scale replicate (AllReduce is long done; no PE stall)
            sh, hh, nhh = finalize_scale(toth_g, 1.0 / (D * VOCAB), "hd")
            qd = drp.tile([NCH_HD, 128, DT * 256], F16, tag="qd")

            # hoist the first B1 weight chunks' quantize ahead of the LN
            # DVE work so the PE can restart on B1 right after h3 lands
            NPRE = 2
            preq = []
            for g in range(NPRE):
                wt = wstr.tile([128, DT, 256], F32, tag="wstr")
                dma2(wt[:], hwt_d.ap()[g], DT // 2)
                q = quantize(wt[:], hh, nhh)
                dma2(qd[g], q[:].rearrange("p dt c -> p (dt c)"), DT * 128)
                preq.append(q)

            # ---- layernorm scalars ----
            mu = sml.tile([1, T], F32, tag="mu")
            nc.scalar.activation(mu[:], ps_s[:], AF.Copy, scale=1.0 / D)
            ms = sml.tile([1, T], F32, tag="ms")
            nc.scalar.activation(ms[:], ps_q[:], AF.Copy, scale=1.0 / D)
            var = sml.tile([1, T], F32, tag="var")
            nc.vector.tensor_tensor(var[:], mu[:], mu[:], OP.mult)
            nc.vector.tensor_tensor(var[:], ms[:], var[:], OP.subtract)
            eps1 = cst.tile([1, 1], F32)
            nc.any.memset(eps1[:], EPS)
            sd = sml.tile([1, T], F32, tag="sd")
            nc.scalar.activation(sd[:], var[:], AF.Sqrt, bias=eps1[:])
            rstd = sml.tile([1, T], F32, tag="rstd")
            nc.vector.reciprocal(rstd[:], sd[:])
            negmur = sml.tile([1, T], F32, tag="r0")
            nc.vector.tensor_tensor(negmur[:], mu[:], rstd[:], OP.mult)
            nc.vector.tensor_scalar(negmur[:], negmur[:], -1.0, None, OP.mult)
            # broadcast to [128, T] via ones-matmul
            pa = ps_st.tile([128, T], F32, tag="pa")
            nc.tensor.matmul(pa[:], ones_row[:], rstd[:], start=True, stop=True)
            a_b = cst.tile([128, T], F32)
            nc.scalar.activation(a_b[:], pa[:], AF.Copy)
            pb = ps_st.tile([128, T], F32, tag="pa")
            nc.tensor.matmul(pb[:], ones_row[:], negmur[:], start=True, stop=True)
            b_b = cst.tile([128, T], F32)
            nc.scalar.activation(b_b[:], pb[:], AF.Copy)

            # ---- LN transform -> h3 fp16 ----
            h3 = act.tile([128, DT, T], ADT, tag="act")
            for dt in range(DT):
                if cfg.identity_ln:
                    t1 = evt.tile([128, T], F32, tag="evt32")
                    nc.vector.tensor_tensor(t1[:], y1[:, dt, :], a_b[:], OP.mult)
                    nc.vector.tensor_tensor(h3[:, dt, :], t1[:], b_b[:], OP.add)
                else:
                    t1 = evt.tile([128, T], F32, tag="evt32")
                    nc.vector.tensor_tensor(t1[:], y1[:, dt, :], a_b[:], OP.mult)
                    nc.vector.tensor_tensor(t1[:], t1[:], b_b[:], OP.add)
                    nc.vector.tensor_scalar(h3[:, dt, :], t1[:],
                                            gams[:, dt:dt + 1],
                                            bets[:, dt:dt + 1], OP.mult, OP.add)

            # ---- AllGather LN outputs across the 8 cores, split into two
            # token-halves so head compute on half A starts while half B is
            # still in flight ----
            TH = T // 2
            agins, agouts = [], []
            for hh_i in range(2):
                agin = drp.tile([128, DT * TH], F16, tag=f"agin{hh_i}",
                                name=f"agin{hh_i}")
                dma2(agin[:].rearrange("p (dt t) -> p dt t", dt=DT),
                     h3[:, :, hh_i * TH:(hh_i + 1) * TH], DT // 2)
                agins.append(agin)
            agouts = []
            for hh_i in range(2):
                agout = drp.tile([NBLK, 128, DT * TH], F16, tag=f"agout{hh_i}",
                                 name=f"agout{hh_i}", addr_space="Shared")
                nc.gpsimd.collective_compute(
                    "AllGather", OP.bypass,
                    replica_groups=[list(range(cfg.ncores))],
                    ins=[agins[hh_i][:].opt()], outs=[agout[:].opt()])
                agouts.append(agout)

            # ---- head phase 1: own block (h3 in SBUF) while AllGather runs.
            # Quantized chunks are cached to DRAM for phase 2.
            # out block column 0 = own tokens; column j = global block (pid+j)%8
            def head_group(rhs_blocks, tw, wcols, qsrc):
                # rhs_blocks: per-block fn(dt)->[128,tw]; wcols: list of
                # (half, colstart, stage_lo, stage_n) write descriptors
                nbl = len(rhs_blocks)
                for g in range(NCH_HD):
                    if qsrc is None and g < NPRE:
                        q = preq[g]
                    elif qsrc is None:
                        wt = wstr.tile([128, DT, 256], F32, tag="wstr")
                        dma2(wt[:], hwt_d.ap()[g], DT // 2)
                        q = quantize(wt[:], hh, nhh)
                        dma2(qd[g], q[:].rearrange("p dt c -> p (dt c)"),
                             DT * 128)
                    else:
                        q = qbuf.tile([128, DT, 256], ADT, tag="sgn", name="qld")
                        dma2(q[:].rearrange("p dt c -> p (dt c)"), qsrc[g],
                             DT * 128)
                    for j in range(2):
                        ot = g * 2 + j
                        stage = osb.tile([128, BPH * T], F16, tag="osb")
                        pts = [ps_mm.tile([128, tw], F32, tag="ps_mm",
                                          name=f"pt{bj}")
                               for bj in range(nbl)]
                        for dt in range(DT):
                            lhsT = q[:, dt, j * 128:(j + 1) * 128]
                            for bj in range(nbl):
                                nc.tensor.matmul(pts[bj][:], lhsT,
                                                 rhs_blocks[bj](dt),
                                                 start=(dt == 0),
                                                 stop=(dt == DT - 1))
                        for bj in range(nbl):
                            nc.scalar.activation(stage[:, bj * tw:(bj + 1) * tw],
                                                 pts[bj][:], AF.Identity,
                                                 bias=hbs[:, ot:ot + 1], scale=sh[:])
                        for (whalf, wcol, slo, sn) in wcols:
                            nc.sync.dma_start(
                                out_d.ap()[ot, whalf][:, wcol:wcol + sn],
                                stage[:, slo:slo + sn])

            TQ = T // 2
            head_group([lambda dt: h3[:, dt, :]], T,
                       [(0, 0, 0, TQ), (1, 0, TQ, TQ)], None)

            # ---- head phase 2: remote blocks (pid+1 .. pid+7) x token-half,
            # dynamic reads from the gathered halves ----
            pid = nc.sync.partition_id()
            for hh_i in range(2):
                done = 1
                for grp in [2, 2, 2, 1]:
                    acts = blk.tile([128, DT, 2, TH], ADT, tag="blk",
                                    name="acts")
                    for j in range(grp):
                        bidx = (pid + done + j) % NBLK
                        nc.sync.dma_start(
                            acts[:, :, j, :],
                            agouts[hh_i][bass.ds(bidx, 1)][0].rearrange(
                                "p (dt t) -> p dt t", dt=DT))
                    head_group(
                        [(lambda jj: (lambda dt: acts[:, dt, jj, :]))(j)
                         for j in range(grp)],
                        TH,
                        [(hh_i, done * TH, 0, grp * TH)],
                        qd)
                    done += grp

    nc.compile()
    return nc


_BUILD_CACHE = {}


def _get_nc(cfg: Cfg):
    key = (cfg.ncores, cfg.adt, cfg.halves, cfg.identity_ln)
    if key not in _BUILD_CACHE:
        _BUILD_CACHE[key] = build(cfg)
    return _BUILD_CACHE[key]


def make_in_maps(cfg: Cfg, x, emb, w0, b0, w1, b1, ln_gamma, ln_beta, head_w, head_b):
    """Host-side sharding/layout prep. Returns list of per-core input dicts."""
    D, T, VS = cfg.D, cfg.T, cfg.VS
    embh = np.asarray(emb, np.float32).astype(np.float16)

    def tile_w(wT, ncols):
        # [D, ncols] (transposed weight) -> [ncols//256, 128, DT, 256] contiguous
        return np.ascontiguousarray(
            wT.reshape(cfg.DT, 128, ncols // 256, 256).transpose(2, 1, 0, 3))

    def flat16(wT, sc=4096):
        w = wT.astype(np.float16).reshape(-1)
        return np.ascontiguousarray(w.reshape(-1, 128, sc))

    w0T = np.ascontiguousarray(np.asarray(w0, np.float32).T)
    w1T = np.ascontiguousarray(np.asarray(w1, np.float32).T)
    w0t = tile_w(w0T, D)
    w1t = tile_w(w1T, D)
    w0h = flat16(w0T)
    w1h = flat16(w1T)

    hw = np.asarray(head_w, np.float32)
    hw_pad = np.zeros((VPAD, D), np.float32)
    hw_pad[:VOCAB] = hw
    hwT = np.ascontiguousarray(hw_pad.T)          # [D, VPAD]
    hb_pad = np.zeros((VPAD,), np.float32)
    hb_pad[:VOCAB] = np.asarray(head_b, np.float32)

    def rearr(v, n):
        return np.ascontiguousarray(np.asarray(v, np.float32).reshape(n, 128).T)

    b0r = rearr(b0, D // 128)
    b1r = rearr(b1, D // 128)
    gamr = rearr(ln_gamma, D // 128)
    betr = rearr(ln_beta, D // 128)

    ids = np.asarray(x).reshape(-1).astype(np.int16)
    assert ids.size == cfg.ncores * T
    in_maps = []
    for c in range(cfg.ncores):
        idx_arr = np.tile(ids[c * T:(c + 1) * T].reshape(T // 16, 16).T, (8, 1))
        hwT_c = np.ascontiguousarray(hwT[:, c * VS:(c + 1) * VS])
        hbr_c = rearr(hb_pad[c * VS:(c + 1) * VS], VS // 128)
        in_maps.append(dict(
            idx=idx_arr, embh=embh, w0t=w0t, w1t=w1t, w0h=w0h, w1h=w1h,
            hwt=tile_w(hwT_c, VS), hwh=flat16(hwT_c),
            b0r=b0r, b1r=b1r, gamr=gamr, betr=betr, hbr=hbr_c))
    return in_maps


def _run(cfg: Cfg, inputs, trace=False, tmpdir=None):
    nc = _get_nc(cfg)
    in_maps = make_in_maps(cfg, **inputs)
    res = run_bass_kernel_spmd(nc, in_maps, core_ids=list(range(cfg.ncores)),
                               trace=trace, tmpdir=tmpdir)
    TOK = cfg.ncores * cfg.T
    TQ = cfg.T // 2
    outs = []
    for c in range(cfg.ncores):
        # out[ot, half, p, j*TQ+t]: block col j = global block (c+j)%8,
        # local token = half*TQ + t
        o = res.results[c]["out"].reshape(cfg.NO_HD, 2, 128, cfg.ncores, TQ)
        o = np.roll(o, c, axis=3)          # -> [ot, half, p, gblk, t]
        o = o.transpose(0, 2, 3, 1, 4)     # -> [ot, p, gblk, half, t]
        outs.append(o.reshape(cfg.VS, TOK))
    full = np.concatenate(outs, axis=0)[:VOCAB]   # [V, 4096]
    return np.ascontiguousarray(full.T).astype(np.float32), res


def kernel(**inputs) -> np.ndarray:
    ident = (np.all(np.asarray(inputs["ln_gamma"]) == 1.0)
             and np.all(np.asarray(inputs["ln_beta"]) == 0.0))
    cfg = Cfg(identity_ln=bool(ident))
    full, _ = _run(cfg, inputs)
    return full.reshape(BATCH, SEQ, VOCAB)
